# revision 1
# baseline (speedup 1.0000x reference)
"""Two-layer GAT (PyG GATConv semantics) on 8 Trainium2 NeuronCores.

Strategy (dst-sharded, round-based edge layout, degree-sorted nodes):
- Host: compute in-degrees (incl. self-loop), sort nodes by in-degree
  descending, deal nodes round-robin to cores (rank r -> core r%8,
  slot r//8).  Each core owns 6250 destination slots = 49 tiles of 128.
  Within a tile, partition j = dst slot, round r = r-th incoming edge.
  Because tiles group nodes of nearly-equal degree, the per-tile round
  count (max degree in tile) wastes only ~2.5% padding.  Padding rounds
  point at a dummy table row with a_s = -80 (inside the HW exp table's
  valid range, unlike -1e30), making the edge weight ~1e-7 and h = 0.
- Device, per layer: project node features (h = x @ W with folded
  attention-logit columns and skip projection, bf16 PE matmuls; the
  layer-1 input is host-pre-transposed so no PE transpose is needed),
  AllGather the node-row table [h(128)|a_s(2)|pad(2)] bf16 (stored as
  f32 tensors of half the columns — bf16 collectives mis-place blocks),
  issued in 7 chunks overlapped with projection (chunk-major table
  layout keeps each chunk's output contiguous).  Per dst tile: round 0
  (the self-loop) is a direct DMA of the core's own cc rows; remaining
  rounds are 128-row indirect DMAs ([128,1] offset columns — the only
  offset shape this stack's SWDGE ucode supports; multi-column offsets
  and InstDMAGatherAnt produce garbage / hang without the mlp Q7
  library, which cannot be loaded through this walrus).  Per-edge
  weights w = max(exp(z), exp(0.2 z)) with z = a_s + a_d via broadcast
  vector ops (exp(leaky_relu) factorization; softmax scale-invariance
  makes max-subtraction unnecessary), aggregation via one broadcast
  multiply + strided tensor_reduce over rounds, then normalize, add
  skip, relu.  A 3-stage software pipeline (gather/attend k | aggregate
  k-1 | layer-2 projection tail k-2) keeps every engine's instruction
  stream unblocked; the kernel is bound by gpsimd SWDGE descriptor
  generation (~1.1 us per 128-row indirect DMA).
"""

import sys

if "/opt/trn_rl_repo" not in sys.path:
    sys.path.insert(0, "/opt/trn_rl_repo")

import numpy as np

import concourse.bass as bass
import concourse.mybir as mybir
import concourse.tile as tile
from concourse.bass_utils import run_bass_kernel_spmd
from concourse.masks import make_identity

N, E, F_IN, H, C = 50000, 800000, 128, 2, 64
HC = H * C
NCORES = 8
SHARD = N // NCORES            # 6250
P = 128
TILES = (SHARD + P - 1) // P   # 49
NPAD = TILES * P               # 6272
ROW = 132                      # table row: h(128) | a_s(2) | pad(2)
PRJ = 260                      # proj cols: W(128) | w_as(2) | w_ad(2) | Wsk(128)
NTAB = NCORES * NPAD           # table rows for real+pad slots (50176)
DUMMY_ZERO = NTAB              # spare: h=0, a_s=0
DUMMY_NEG = NTAB + 1           # padding row: h=0, a_s=-80 (w ~ 1e-7)
TBL = NTAB + 2
AGCH = 7                       # target AllGather chunk count
NCHUNK = AGCH if TILES % AGCH == 0 else 1
TPC = TILES // NCHUNK          # tiles per AG chunk
CHROWS = TPC * P               # cc rows per AG chunk
# table layout is chunk-major so each chunk's AllGather output is a
# contiguous row range: row(n) = q*8*CHROWS + core*CHROWS + slot%CHROWS

F32 = mybir.dt.float32
BF16 = mybir.dt.bfloat16
I32 = mybir.dt.int32
NP_BF16 = mybir.dt.np(BF16)


def _split_sync_waits(nc, limit=1):
    """walrus in this container rejects >1 sync wait per instruction; move
    excess waits onto NoOps inserted just before the offending one."""
    ctr = [0]

    def fresh_noop(engine, waits):
        ctr[0] += 1
        return mybir.InstNoOp(
            name=f"waitsplit-{ctr[0]}",
            engine=engine,
            bass_nofuse=True,
            sync_info=mybir.SyncInfo(on_wait=list(waits), on_update=[]),
        )

    for f in nc.m.functions:
        for bb in f.blocks:
            out = []
            changed = False
            for ins in bb.instructions:
                si = ins.sync_info
                waits = list(si.on_wait) if si else []
                if len(waits) > limit:
                    changed = True
                    excess, keep = waits[:-limit], waits[-limit:]
                    for i in range(0, len(excess), limit):
                        noop = fresh_noop(ins.engine, excess[i : i + limit])
                        nc.register_instruction(noop, overwrite=True)
                        out.append(noop)
                    ins.sync_info = mybir.SyncInfo(
                        on_wait=keep, on_update=list(si.on_update)
                    )
                out.append(ins)
            if changed:
                bb.instructions = out
    return ctr[0]


def _host_prep(src, dst):
    s = src.astype(np.int64)
    d = dst.astype(np.int64)
    # degree stats include the implicit self-loop; the self-loop itself is
    # served by a direct DMA of the core's own cc rows (round 0), so gather
    # lists cover only the E original edges.
    indeg = np.bincount(d, minlength=N) + 1
    order = np.argsort(-indeg, kind="stable")         # node of rank r
    rank = np.empty(N, np.int64)
    rank[order] = np.arange(N)
    slot_of = rank // NCORES
    pos_of = (
        (slot_of // CHROWS) * (NCORES * CHROWS)
        + (rank % NCORES) * CHROWS
        + slot_of % CHROWS
    )                                                 # table row per node

    indeg_sorted = indeg[order]
    R = np.empty(TILES, np.int64)                     # indirect rounds/tile
    for k in range(TILES):
        R[k] = indeg_sorted[k * P * NCORES : (k + 1) * P * NCORES].max() - 1
    OFF = np.concatenate([[0], np.cumsum(R)]).astype(np.int64)
    TOTR = int(OFF[-1])

    er = rank[d]                                      # dst rank per edge
    # secondary sort by src table position: each round's 128 reads then hit
    # mostly-ascending HBM addresses, cutting DMA completion latency
    eorder = np.lexsort((pos_of[s], er))
    s_s = s[eorder]
    er_s = er[eorder]
    cnt = np.bincount(er, minlength=N)
    starts = np.concatenate([[0], np.cumsum(cnt)])
    seq = np.arange(len(er_s)) - starts[er_s]         # round per edge
    ce = er_s % NCORES
    slot_e = er_s // NCORES
    ke = slot_e // P
    je = slot_e % P

    idx_all = np.full((NCORES, P, max(TOTR, 1)), DUMMY_NEG, np.int32)
    idx_all[ce, je, OFF[ke] + seq] = pos_of[s_s]
    return R, OFF, TOTR, idx_all, order


def _fold_weights(W, att_src, att_dst, Wsk):
    w_as = np.stack([W[:, h * C:(h + 1) * C] @ att_src[h] for h in range(H)], 1)
    w_ad = np.stack([W[:, h * C:(h + 1) * C] @ att_dst[h] for h in range(H)], 1)
    full = np.concatenate([W, w_as, w_ad, Wsk], axis=1).astype(np.float32)
    return full.astype(NP_BF16)


def _build_nc(R, OFF, TOTR):
    nc = bass.Bass(
        "TRN2",
        num_devices=NCORES,
        use_seq_codegen=True,
        dynamic_dma_scratch_size=131072,
    )
    xsT = nc.dram_tensor("xsT", [F_IN, NPAD], BF16, kind="ExternalInput")
    idx = nc.dram_tensor("idx", [P, TOTR], I32, kind="ExternalInput")
    wall1 = nc.dram_tensor("wall1", [F_IN, PRJ], BF16, kind="ExternalInput")
    wall2 = nc.dram_tensor("wall2", [HC, PRJ], BF16, kind="ExternalInput")
    bb1 = nc.dram_tensor("bb1", [P, HC], F32, kind="ExternalInput")
    bb2 = nc.dram_tensor("bb2", [P, HC], F32, kind="ExternalInput")
    spec = nc.dram_tensor("spec", [2, ROW], BF16, kind="ExternalInput")
    out = nc.dram_tensor("out", [SHARD, HC], F32, kind="ExternalOutput")

    # cc/tb declared as f32 (half the columns) because the collective
    # mis-places blocks for bf16 tensors; compute views bitcast to bf16
    layers = []
    for li in (1, 2):
        cc = nc.dram_tensor(f"cc{li}", [NPAD, ROW // 2], F32, kind="Internal")
        tb = nc.dram_tensor(
            f"tb{li}", [TBL, ROW // 2], F32, kind="Internal",
            addr_space="Shared",
        )
        layers.append((cc, tb))

    with tile.TileContext(nc) as tc:
        with (
            tc.tile_pool(name="const", bufs=1) as constp,
            tc.tile_pool(name="proj", bufs=6) as projp,
            tc.tile_pool(name="pjpsum", bufs=3, space="PSUM") as pjpsum,
            tc.tile_pool(name="tppsum", bufs=2, space="PSUM") as tppsum,
            tc.tile_pool(name="p2psum", bufs=3, space="PSUM") as p2psum,
            tc.tile_pool(name="gath", bufs=3) as gathp,
            tc.tile_pool(name="small", bufs=3) as smallp,
            tc.tile_pool(name="fwp", bufs=2) as fwp,
            tc.tile_pool(name="finp", bufs=2) as finp,
        ):
            ident = constp.tile([P, P], BF16)
            make_identity(nc, ident[:])
            walls = {}
            bbs = {}
            for li, wsrc, bsrc in ((1, wall1, bb1), (2, wall2, bb2)):
                wt = constp.tile([P, PRJ], BF16, tag=f"wall{li}")
                nc.sync.dma_start(out=wt[:], in_=wsrc[:])
                bt = constp.tile([P, HC], F32, tag=f"bb{li}")
                nc.sync.dma_start(out=bt[:], in_=bsrc[:])
                walls[li] = wt
                bbs[li] = bt
            spect = constp.tile([2, ROW], BF16, tag="spec")
            nc.sync.dma_start(out=spect[:], in_=spec[:])
            it_all = constp.tile([P, TOTR], I32, tag="itall")
            nc.sync.dma_start(out=it_all[:], in_=idx[:])
            for li in (1, 2):
                nc.sync.dma_start(
                    out=layers[li - 1][1][NTAB:TBL, :],
                    in_=spect[:].bitcast(F32),
                )
            # persistent per-layer state computed by proj, consumed by sweep
            ads = {}
            skls = {}
            for li in (1, 2):
                ad_t = constp.tile([P, TILES, 2], F32, tag=f"ad{li}")
                skl_t = constp.tile([P, TILES, HC], BF16, tag=f"skl{li}")
                ads[li] = ad_t
                skls[li] = skl_t

            def proj_tile(li, k, lhsT):
                """lhsT: [feat, node] bf16 tile (already transposed)."""
                cc, _ = layers[li - 1]
                pj = pjpsum.tile([P, PRJ], F32, tag="pj")
                nc.tensor.matmul(
                    out=pj[:], lhsT=lhsT[:], rhs=walls[li][:],
                    start=True, stop=True,
                )
                rowst = projp.tile([P, ROW], BF16, tag="rowst")
                nc.vector.tensor_copy(out=rowst[:], in_=pj[:, 0:132])
                nc.sync.dma_start(
                    out=cc[k * P : (k + 1) * P, :],
                    in_=rowst[:, :].bitcast(F32),
                )
                nc.scalar.copy(out=ads[li][:, k, :], in_=pj[:, 130:132])
                nc.vector.tensor_add(
                    out=skls[li][:, k, :], in0=pj[:, 132:260], in1=bbs[li][:]
                )

            def collective_chunk(li, i):
                cc, tb = layers[li - 1]
                nc.gpsimd.collective_compute(
                    "AllGather",
                    mybir.AluOpType.bypass,
                    replica_groups=[list(range(NCORES))],
                    ins=[cc[i * CHROWS : (i + 1) * CHROWS, :]],
                    outs=[
                        tb[
                            i * NCORES * CHROWS : (i + 1) * NCORES * CHROWS,
                            :,
                        ]
                    ],
                )



            def stage_a(li, k, state):
                """Gather tile k's rows + compute per-edge z and exps.
                Round 0 is the self-loop, loaded directly from this core's
                own cc rows (contiguous — no gpsimd descriptor cost)."""
                cc, tb = layers[li - 1]
                Rk = int(R[k]) + 1                   # total rounds incl. self
                off = int(OFF[k])
                gt = gathp.tile([P, Rk, ROW], BF16, tag="gt")
                nc.scalar.dma_start(
                    out=gt[:, 0, :].bitcast(F32),
                    in_=cc[k * P : (k + 1) * P, :],
                )
                for r in range(Rk - 1):
                    nc.gpsimd.indirect_dma_start(
                        out=gt[:, 1 + r, :].bitcast(F32),
                        out_offset=None,
                        in_=tb[:],
                        in_offset=bass.IndirectOffsetOnAxis(
                            ap=it_all[:, off + r : off + r + 1], axis=0
                        ),
                    )
                z = smallp.tile([P, Rk, 2], F32, tag="z")
                nc.vector.tensor_tensor(
                    out=z[:],
                    in0=gt[:, :, 128:130],
                    in1=ads[li][:, k, :]
                    .rearrange("p h -> p () h")
                    .to_broadcast([P, Rk, 2]),
                    op=mybir.AluOpType.add,
                )
                e1 = smallp.tile([P, Rk, 2], F32, tag="e1")
                nc.scalar.activation(
                    out=e1[:], in_=z[:], func=mybir.ActivationFunctionType.Exp
                )
                e2 = smallp.tile([P, Rk, 2], F32, tag="e2")
                nc.scalar.activation(
                    out=e2[:], in_=z[:],
                    func=mybir.ActivationFunctionType.Exp, scale=0.2,
                )
                state[k] = (gt, e1, e2)

            def stage_b(li, k, state, pstate):
                """Aggregate tile k; layer 1 starts the layer-2 projection."""
                gt, e1, e2 = state.pop(k)
                Rk = int(R[k]) + 1
                rows = min(P, SHARD - k * P)
                w = smallp.tile([P, Rk, 2], BF16, tag="w")
                nc.vector.tensor_tensor(
                    out=w[:], in0=e1[:], in1=e2[:], op=mybir.AluOpType.max
                )
                fw = fwp.tile([P, Rk, HC], BF16, tag="fw")
                nc.vector.tensor_tensor(
                    out=fw[:].rearrange("p r (h c) -> p r h c", h=H),
                    in0=gt[:, :, 0:HC].rearrange("p r (h c) -> p r h c", h=H),
                    in1=w[:]
                    .rearrange("p r h -> p r h ()")
                    .to_broadcast([P, Rk, H, C]),
                    op=mybir.AluOpType.mult,
                )
                acc = finp.tile([P, HC], F32, tag="acc")
                nc.vector.tensor_reduce(
                    out=acc[:],
                    in_=fw[:].rearrange("p r c -> p c r"),
                    axis=mybir.AxisListType.X,
                    op=mybir.AluOpType.add,
                )
                wsum = finp.tile([P, 2], F32, tag="wsum")
                nc.vector.tensor_reduce(
                    out=wsum[:],
                    in_=w[:].rearrange("p r h -> p h r"),
                    axis=mybir.AxisListType.X,
                    op=mybir.AluOpType.add,
                )
                rec = finp.tile([P, 2], F32, tag="rec")
                nc.vector.reciprocal(out=rec[:], in_=wsum[:])
                ot = finp.tile([P, HC], F32, tag="ot")
                nc.vector.tensor_tensor(
                    out=ot[:].rearrange("p (h c) -> p h c", h=H),
                    in0=acc[:].rearrange("p (h c) -> p h c", h=H),
                    in1=rec[:]
                    .rearrange("p h -> p h ()")
                    .to_broadcast([P, H, C]),
                    op=mybir.AluOpType.mult,
                )
                ot2 = finp.tile([P, HC], F32, tag="ot2")
                nc.vector.tensor_tensor(
                    out=ot2[:], in0=ot[:], in1=skls[li][:, k, :],
                    op=mybir.AluOpType.add,
                )
                if li == 1:
                    # relu, then feed layer-2 projection (transpose on PE)
                    xt2 = finp.tile([P, HC], BF16, tag="xt2")
                    nc.scalar.activation(
                        out=xt2[:], in_=ot2[:],
                        func=mybir.ActivationFunctionType.Relu,
                    )
                    tp = tppsum.tile([P, P], BF16, tag="tp")
                    nc.tensor.transpose(
                        out=tp[:], in_=xt2[:], identity=ident[:]
                    )
                    xT2 = projp.tile([P, P], BF16, tag="xT2")
                    nc.scalar.copy(out=xT2[:], in_=tp[:])
                    pj2 = p2psum.tile([P, PRJ], F32, tag="pj2")
                    nc.tensor.matmul(
                        out=pj2[:], lhsT=xT2[:], rhs=walls[2][:],
                        start=True, stop=True,
                    )
                    pstate[k] = pj2
                else:
                    nc.sync.dma_start(
                        out=out[k * P : k * P + rows, :], in_=ot2[:rows, :]
                    )

            def stage_c(k, pstate):
                """Layer-2 projection tail for tile k (from PSUM pj2)."""
                pj2 = pstate.pop(k)
                cc2, _ = layers[1]
                rowst = projp.tile([P, ROW], BF16, tag="rowst")
                nc.vector.tensor_copy(out=rowst[:], in_=pj2[:, 0:132])
                nc.sync.dma_start(
                    out=cc2[k * P : (k + 1) * P, :],
                    in_=rowst[:, :].bitcast(F32),
                )
                nc.scalar.copy(out=ads[2][:, k, :], in_=pj2[:, 130:132])
                nc.vector.tensor_add(
                    out=skls[2][:, k, :], in0=pj2[:, 132:260], in1=bbs[2][:]
                )
                if (k + 1) % TPC == 0:
                    collective_chunk(2, (k + 1) // TPC - 1)

            for k in range(TILES):
                xt = projp.tile([P, P], BF16, tag="xt")
                nc.scalar.dma_start(
                    out=xt[:], in_=xsT[:, k * P : (k + 1) * P]
                )
                proj_tile(1, k, xt)
                if (k + 1) % TPC == 0:
                    collective_chunk(1, (k + 1) // TPC - 1)
            # 3-stage pipelined sweep: A(k) | B(k-1) | C(k-2) so no engine
            # sits behind another engine's in-flight work
            st1, ps1 = {}, {}
            stage_a(1, 0, st1)
            for k in range(1, TILES):
                stage_a(1, k, st1)
                stage_b(1, k - 1, st1, ps1)
                if k >= 2:
                    stage_c(k - 2, ps1)
            stage_b(1, TILES - 1, st1, ps1)
            stage_c(TILES - 2, ps1)
            stage_c(TILES - 1, ps1)
            st2 = {}
            stage_a(2, 0, st2)
            for k in range(1, TILES):
                stage_a(2, k, st2)
                stage_b(2, k - 1, st2, None)
            stage_b(2, TILES - 1, st2, None)

    _split_sync_waits(nc, limit=1)
    return nc


_CACHE = {}


def _get_program(src, dst):
    key = (hash(src.tobytes()), hash(dst.tobytes()))
    if key not in _CACHE:
        R, OFF, TOTR, idx_all, order = _host_prep(src, dst)
        nc = _build_nc(R, OFF, TOTR)
        _CACHE[key] = (nc, idx_all, order)
    return _CACHE[key]


def _prepare_inputs(inputs, idx_all, order):
    x = np.asarray(inputs["x"], np.float32)
    wall1 = _fold_weights(
        np.asarray(inputs["W1"]), np.asarray(inputs["att_src1"]),
        np.asarray(inputs["att_dst1"]), np.asarray(inputs["Wsk1"]),
    )
    wall2 = _fold_weights(
        np.asarray(inputs["W2"]), np.asarray(inputs["att_src2"]),
        np.asarray(inputs["att_dst2"]), np.asarray(inputs["Wsk2"]),
    )
    bb1 = np.tile(
        (np.asarray(inputs["b1"]) + np.asarray(inputs["bsk1"]))[None, :], (P, 1)
    ).astype(np.float32)
    bb2 = np.tile(
        (np.asarray(inputs["b2"]) + np.asarray(inputs["bsk2"]))[None, :], (P, 1)
    ).astype(np.float32)
    # padding-row a_s: very negative but inside the HW exp table's valid
    # input range (exp(-1e30) returns garbage on the ACT engine).
    # w_pad = exp(leaky_relu(-80 + a_d)) <= exp(-16 + 0.2*a_d) ~ 1e-7,
    # negligible vs real softmax denominators; h=0 keeps numerators exact.
    spec = np.zeros((2, ROW), NP_BF16)
    spec[1, 128:130] = np.float32(-80.0)

    in_maps = []
    for c in range(NCORES):
        xsTv = np.zeros((F_IN, NPAD), NP_BF16)
        xsTv[:, :SHARD] = x[order[c::NCORES]].T.astype(NP_BF16)
        in_maps.append(
            {
                "xsT": xsTv,
                "idx": idx_all[c],
                "wall1": wall1,
                "wall2": wall2,
                "bb1": bb1,
                "bb2": bb2,
                "spec": spec,
            }
        )
    return in_maps


def _unshard(results, order):
    outp = np.empty((N, HC), np.float32)
    for c in range(NCORES):
        outp[order[c::NCORES]] = results[c]["out"]
    return outp


def _run(inputs, trace=False):
    src = np.asarray(inputs["src"])
    dst = np.asarray(inputs["dst"])
    nc, idx_all, order = _get_program(src, dst)
    in_maps = _prepare_inputs(inputs, idx_all, order)
    res = run_bass_kernel_spmd(
        nc, in_maps, core_ids=list(range(NCORES)), trace=trace
    )
    outp = _unshard(res.results, order)
    return outp, res.exec_time_ns


def kernel(**inputs) -> np.ndarray:
    out, _ = _run(inputs, trace=False)
    return out


def kernel_traced(**inputs):
    return _run(inputs, trace=True)

